# revision 35
# baseline (speedup 1.0000x reference)
"""NT-Xent loss kernel for Trainium2, SPMD across 8 NeuronCores.

Strategy (symmetric/circulant decomposition, ~half the exp work):
  - Host precomputes xn = x/||x|| (f32), scales by SCALE and quantizes to
    fp8-e4m3 in the DoubleRow transposed layout [128, 2, N] (k = s*128+p).
  - sim is symmetric, so only block-distances d = (jblk - iblk) mod 64 in
    {0..32} (128-wide blocks) are computed. Rows are owned interleaved:
    core c owns m-tile rows t = 8*mt + c. Each m-tile processes the
    contiguous circulant window of 33 blocks = 4224 cols. Entries with
    d in {1..31} also serve as the transposed entries via column sums;
    d=0 (diagonal block) and d=32 (self-paired distance, computed twice
    globally) contribute row sums only.
  - Host ships, per core, a rotated+extended matrix xe (ext col j <->
    global col (128c + j) mod 8192) so the device program is identical
    across cores, plus the core's own 8 m-tile rows for the stationary
    operand.
  - Device: fp8 DoubleRow matmuls -> fused exp+row-sum (ACT accum_out),
    exp output (bf16) folded into a column-sum accumulator on the vector
    engine; partition-reduction of column sums via ones-matmuls on the
    tensor engine. Outputs per core: [128, 25] row-sum partials and
    [128, 87] column-sum partials.
  - Host finishes: rowsum_total, loss = (sum log(rowsum) - target)/N.
"""

import sys

sys.path.insert(0, "/opt/trn_rl_repo")

from contextlib import ExitStack

import numpy as np

import concourse.bass as bass
import concourse.tile as tile
from concourse import bacc, bass_utils, mybir

F32 = mybir.dt.float32
F8 = mybir.dt.float8e4
BF16 = mybir.dt.bfloat16
AF = mybir.ActivationFunctionType
ALU = mybir.AluOpType
DR = mybir.MatmulPerfMode.DoubleRow

N, D = 8192, 256
NCORES = 8
SHARD = N // NCORES  # 1024 rows per core
MT = SHARD // 128  # 8 m-tiles per core
KT = 2  # two 128-row k-subtiles (D=256), packed via DoubleRow
NBLK = 33  # circulant window: block distances 0..32
WIN = NBLK * 128  # 4224 cols per m-tile window
EXTN = (MT - 1) * 1024 + WIN  # 11392 extended cols
SPAN = WIN // 3  # 1408: ACT span (3 PSUM banks)
NSPAN = 3
# m-tiles 4..7 (rows t>=32) process d in {0..32} and contribute their
# d=32 block to the column sums; m-tiles 0..3 (rows t<32) only need
# d in {0..31} (their d=32 entries arrive via those column sums).
SPANS = ((0, SPAN), (SPAN, 2 * SPAN), (2 * SPAN, WIN))
SPANS_LOW = ((0, SPAN), (SPAN, 2 * SPAN), (2 * SPAN, WIN - 128))
SPANS0 = ((0, 512), (512, SPAN), (SPAN, 2 * SPAN), (2 * SPAN, WIN - 128))
NACC = len(SPANS0) + 3 * len(SPANS_LOW) + 4 * len(SPANS)  # 25 accum columns
COLW = (MT - 1) * 1024 + WIN - 128  # 11264 col-sum accumulator width
NCHUNK_COL = COLW // 128  # 87 ones-matmul chunks
TEMP = 0.5
INV_TEMP = 1.0 / TEMP
EPS = 1e-8
SCALE = 8.0  # fp8 pre-scale on xn entries
EXP_SCALE = INV_TEMP / (SCALE * SCALE)

_CACHE = {}


def _build():
    nc = bacc.Bacc("TRN2", target_bir_lowering=False, debug=False, num_devices=NCORES)

    # merged input: [mt0 lhsT (128) | XE[0:512) | XL rest (896) | XE rest]
    # so one small leading DMA carries the first matmul span's operands
    xa = nc.dram_tensor("xa", [128, KT, SHARD + EXTN], F8, kind="ExternalInput").ap()
    out = nc.dram_tensor(
        "out", [128, NACC + NCHUNK_COL], F32, kind="ExternalOutput"
    ).ap()

    with tile.TileContext(nc) as tc, ExitStack() as ctx:
        big = ctx.enter_context(tc.tile_pool(name="big", bufs=1))
        io = ctx.enter_context(tc.tile_pool(name="io", bufs=1))
        stats = ctx.enter_context(tc.tile_pool(name="stats", bufs=1))
        eop = ctx.enter_context(tc.tile_pool(name="eop", bufs=3))

        XA = big.tile([128, KT, SHARD + EXTN], F8)
        COL = big.tile([128, COLW], BF16)
        OUT = stats.tile([128, NACC + NCHUNK_COL], F32)
        ones = stats.tile([128, 1], BF16)

        # Input DMAs alternate across the two hardware queues; the scalar
        # queue's issues come first on that engine so its transfers start
        # early, with the exp table load filling the gap before the first
        # real exp. Leading 640-col sync chunk carries the first matmul
        # span's operands.
        for a, b in ((640, 2432), (3840, 5248), (7040, 8832), (10624, SHARD + EXTN)):
            nc.scalar.dma_start(XA[:, :, a:b], xa[:, :, a:b])
        for a, b in ((0, 640), (2432, 3840), (5248, 7040), (8832, 10624)):
            nc.sync.dma_start(XA[:, :, a:b], xa[:, :, a:b])

        # Small memsets first, big COL memset last — COL is not needed
        # until the first fold.
        warm = stats.tile([128, 1], F32)
        wacc = stats.tile([128, 1], F32)
        nc.vector.memset(warm[:], 0.0)
        nc.vector.memset(ones[:], 1.0)
        # Prefetch the exp table set while input DMAs stream.
        nc.scalar.activation(warm[:], warm[:], AF.Exp, accum_out=wacc[:])
        nc.vector.memset(COL[:].bitcast(F32), 0.0)

        with tc.tile_pool(name="mm_psum", bufs=2, space="PSUM") as mm_psum, \
             tc.tile_pool(name="colsum_psum", bufs=1, space="PSUM") as col_psum:
            psC = col_psum.tile([128, NCHUNK_COL], F32)

            # m-tile 0's first span is split so the exp stream starts as
            # soon as the first 512-col DMA chunk lands.
            acc_idx = [0]

            def xcol(j):
                # XE column j -> merged-layout column
                return 128 + j if j < 512 else 1536 + (j - 512)

            def main_mt(mt):
                lo = 0 if mt == 0 else 640 + (mt - 1) * 128
                lhsT = XA[:, :, lo : lo + 128]
                eo = eop.tile([128, WIN], BF16, tag="eo")
                base = mt * 1024
                spans = SPANS0 if mt == 0 else (SPANS_LOW if mt < 4 else SPANS)
                for s0, s1 in spans:
                    w = s1 - s0
                    ps = mm_psum.tile([128, SPAN], F32)
                    for off in range(0, w, 512):
                        cw = min(512, w - off)
                        xc = xcol(base + s0 + off)
                        nc.tensor.matmul(
                            ps[:, off : off + cw],
                            lhsT=lhsT,
                            rhs=XA[:, :, xc : xc + cw],
                            start=True,
                            stop=True,
                            perf_mode=DR,
                        )
                    ai = acc_idx[0]
                    acc_idx[0] += 1
                    nc.scalar.activation(
                        eo[:, s0:s1],
                        ps[:, 0:w],
                        AF.Exp,
                        scale=EXP_SCALE,
                        accum_out=OUT[:, ai : ai + 1],
                    )
                    # col-sum-eligible part (only the d=0 block excluded)
                    f0, f1 = max(s0, 128), s1
                    if f0 < f1:
                        c = base + f0 - 128
                        nc.vector.tensor_add(
                            COL[:, c : c + (f1 - f0)],
                            COL[:, c : c + (f1 - f0)],
                            eo[:, f0:f1],
                        )

            def ones_chunks(ks):
                for k in ks:
                    nc.tensor.matmul(
                        psC[:, k : k + 1],
                        lhsT=COL[:, k * 128 : (k + 1) * 128],
                        rhs=ones[:],
                        start=True,
                        stop=True,
                    )

            # software pipeline: chunks [8mt, 8mt+8) are final once
            # fold(mt, span0) has run (earlier windows' folds precede it in
            # DVE program order); issue them on the PE one m-tile behind.
            for mt in range(MT):
                main_mt(mt)
                if mt >= 1:
                    ones_chunks(range((mt - 1) * 8, mt * 8))
            # window 7 finalizes chunks 56..65 (its span-0 region), then
            # 66..76 (span 1) and 77..86 (span 2)
            ones_chunks(range(56, 66))
            ones_chunks(range(66, 77))
            ones_chunks(range(77, NCHUNK_COL))  # 77..87

            nc.vector.tensor_copy(OUT[:, NACC:], psC[:])

        nc.sync.dma_start(out, OUT[:])

    nc.compile()
    return nc


def _get_nc():
    if "nc" not in _CACHE:
        _CACHE["nc"] = _build()
    return _CACHE["nc"]


def _first_pos(y: np.ndarray) -> np.ndarray:
    """first_pos[i] = first index j with y[j] == y[i]."""
    y = np.asarray(y)
    uniq, first = np.unique(y, return_index=True)
    lookup = {int(v): int(f) for v, f in zip(uniq, first)}
    return np.array([lookup[int(v)] for v in y], dtype=np.int64)


def make_in_maps(x: np.ndarray, y: np.ndarray):
    x = np.asarray(x, dtype=np.float32)
    norm = np.maximum(np.sqrt((x * x).sum(axis=1, keepdims=True)), EPS)
    xn = x / norm

    # target term (exact, f32): sum_i sim[i, first_pos_i]
    fp = _first_pos(y)
    target_total = float((xn * xn[fp]).sum(dtype=np.float64) * INV_TEMP)

    f8 = mybir.dt.np(F8)
    xq = (xn * SCALE).astype(f8)  # [N, D]
    # DoubleRow transposed layout: xfT[p, s, j] = xq[j, s*128 + p]
    xfT = np.ascontiguousarray(xq.T.reshape(KT, 128, N).transpose(1, 0, 2))
    x2 = np.concatenate([xfT, xfT], axis=2)  # wrap-around halo

    in_maps = []
    for c in range(NCORES):
        off = 128 * c
        xe = x2[:, :, off : off + EXTN]
        xl = np.empty((128, KT, SHARD), dtype=f8)
        for mt in range(MT):
            r = (8 * mt + c) * 128
            xl[:, :, mt * 128 : (mt + 1) * 128] = xfT[:, :, r : r + 128]
        xa = np.concatenate(
            [xl[:, :, 0:128], xe[:, :, 0:512], xl[:, :, 128:], xe[:, :, 512:]],
            axis=2,
        )
        in_maps.append({"xa": np.ascontiguousarray(xa)})
    return in_maps, target_total


def run(in_maps, trace=False, **kwargs):
    nc = _get_nc()
    return bass_utils.run_bass_kernel_spmd(
        nc, in_maps, core_ids=list(range(NCORES)), trace=trace, **kwargs
    )


def finish(results, target_total: float) -> np.ndarray:
    rowsum = np.zeros(N, dtype=np.float64)
    for c, r in enumerate(results):
        o = np.asarray(r["out"], dtype=np.float64)  # [128, 26+87]
        a = 0
        for mt in range(MT):
            spans = SPANS0 if mt == 0 else (SPANS_LOW if mt < 4 else SPANS)
            b = a + len(spans)
            base = (8 * mt + c) * 128
            rowsum[base : base + 128] += o[:, a:b].sum(axis=1)
            a = b
        colv = o[:, NACC:]  # [128, 87]; ext col = 128 + 128k + m
        g = (128 * c + 128 + 128 * np.arange(NCHUNK_COL)[None, :]
             + np.arange(128)[:, None]) % N
        np.add.at(rowsum, g, colv)
    lse_sum = np.log(rowsum).sum()
    return np.asarray(np.float32((lse_sum - target_total) / N))


def kernel(x: np.ndarray, y: np.ndarray) -> np.ndarray:
    in_maps, target_total = make_in_maps(x, y)
    res = run(in_maps)
    return finish(res.results, target_total)
